# revision 44
# baseline (speedup 1.0000x reference)
"""ACNN sparse-attention Trainium2 kernel (8 NeuronCores, pure data parallel).

Reference computation (per batch b, token s):
  windows[s, w]   = x[s + w - 2]           (zero-padded outside the sequence)
  q               = x[s] @ Wq
  kp[s']          = x[s'] @ Wk             (zero rows stay zero)
  scores[s, w]    = sum_h v[h] * tanh(kp[s + w - 2, h] + q[s, h])
  att             = softmax_w(scores)
  out[s]          = sum_w att[s, w] * (x[s + w - 2] @ cnn_W[w*F:(w+1)*F]) + b

Sharding: batch 16 -> 2 per core. Everything on-device is computed from
x^T ([F, tokens]) so all matmul contractions sit on the partition dim;
the W=5 window shifts become free-dim column shifts of x^T / kp^T.

v2 changes vs baseline (297us -> ~227us):
  - q/k projections run as fp8e4 DoubleRow matmuls (x and Wq/Wk quantized
    to e4m3, weights pre-scaled by 64; the 1/64 descale folds into the
    tanh's ACT input scale). Halves the qkp PE stream time (55us -> 29us).
  - the att broadcast (the baseline's hidden co-bottleneck: all of its
    fan-out DMAs read SBUF partitions 0-4, which share one 27 GB/s AXI
    port, ~24us/block serialized) is now a 2-hop spread-source broadcast:
    attT's 5 rows are first flattened into 8 replica rows at partition
    stride 8 (8 different AXI ports), then ONE dma fans [8,16,2596] out
    to all 128 partitions reading 8 ports in parallel (~3us/block, 168
    descriptors instead of 640).
  - out stores go out as one [128, 4*H] DMA per block (4KB/partition
    descriptors); the host de-interleaves the (block, partition, group)
    token order. The tail block stores per-group so the last store
    overlaps the final matmuls.
  - weights (wq/wk/cw) and the fp8 x copy are host-prepacked into their
    SBUF layouts so each loads with fat contiguous-per-partition
    descriptors, and the in-loop DMAs (broadcast chain + out) no longer
    share a queue with the input stream (no head-of-line blocking).
"""
import sys

sys.path.insert(0, "/opt/trn_rl_repo")

from contextlib import ExitStack

import numpy as np
import ml_dtypes

import concourse.bass as bass
import concourse.tile as tile
from concourse import mybir
from concourse.bass_utils import run_bass_kernel_spmd

def _install_ntff_hook_shim():
    """The image's antenv package lacks axon_hooks; recreate it so
    run_bass_kernel_spmd(trace=True) can capture NTFF profiles."""
    import types

    if "antenv.axon_hooks" in sys.modules:
        return
    mod = types.ModuleType("antenv.axon_hooks")
    mod._hook = None
    mod.set_axon_ntff_profile_hook = lambda h: setattr(mod, "_hook", h)
    mod.get_axon_ntff_profile_hook = lambda: mod._hook
    sys.modules["antenv.axon_hooks"] = mod
    try:
        sys.path.insert(0, "/root/.axon_site/trn_agent_boot")
        import trn_boot

        hook = trn_boot._ntff_profile_via_ctypes("/opt/axon/libaxon_pjrt.so")
        if hook is not None:
            mod._hook = hook
    except Exception:
        pass


_install_ntff_hook_shim()

F32 = mybir.dt.float32
BF16 = mybir.dt.bfloat16
F8 = mybir.dt.float8e4
AF = mybir.ActivationFunctionType
AX = mybir.AxisListType
DR = mybir.MatmulPerfMode.DoubleRow

B, S, F, H, W = 16, 2048, 512, 512, 5
PAD = W // 2
NCORES = 8
BPC = B // NCORES          # batches per core
T = BPC * S                # tokens per core (4096)
TB = 512                   # token block
NB = T // TB               # 8 blocks per core
NBSEG = S // TB            # 4 blocks per segment (batch)
FC = F // 128              # 4 f-chunks
HC = H // 128              # 4 h-chunks
SEGW = S + 2 * PAD         # padded segment width in SBUF (2052)
XW = BPC * SEGW            # padded token width in SBUF (4104)
WSCALE = 64.0              # fp8 pre-scale on Wq/Wk (descale folds into tanh)

ABW = 520                  # per-w read-window stride in the bcast ab tile (even)
ABST = ABW + 1             # att_w starts at col ABST*w (offset w inside window)
ABF = (W - 1) * ABST + TB  # used width of the flat/ab rows (2596)
FLW = W * ABST             # allocated flat row width (2605)

_MAX_CTRL_WAITS = 1


def _patch_tile_drain():
    """walrus rejects >4 sem waits on one CTRL; spread the TileContext exit
    drain's waits over extra drain instructions."""
    if getattr(tile.TileContext, "_acnn_drain_patched", False):
        return
    from concourse.vector_clock import ScopedClock

    def _drain_and_barrier(self, tick_clock, wait_clock):
        DRAIN_WAITS = 4  # CTRL instructions accept up to 4 sem waits
        nc = self.nc
        drain_inst = nc.sync.drain()
        wait_clock.add_sem_waits(
            drain_inst.ins, ScopedClock({None: tick_clock.global_clock})
        )
        si = drain_inst.ins.sync_info
        waits = list(si.on_wait) if si and si.on_wait else []
        if len(waits) > DRAIN_WAITS:
            si.on_wait = waits[:DRAIN_WAITS]
            rest = waits[DRAIN_WAITS:]
            for i in range(0, len(rest), DRAIN_WAITS):
                extra = nc.sync.drain()
                esi = extra.ins.sync_info
                chunk = rest[i : i + DRAIN_WAITS]
                if esi is None:
                    extra.ins.sync_info = type(si)(on_wait=chunk, on_update=[])
                else:
                    esi.on_wait = chunk
        nc.all_engine_barrier()
        popped = nc._tile_sem_poison_stack.pop()
        assert popped is self._sem_poison
        nc.clear_and_free_semaphores(list(self.sems.allocated().values()))
        nc.all_engine_barrier()

    tile.TileContext._drain_and_barrier = _drain_and_barrier
    tile.TileContext._acnn_drain_patched = True


def _split_multi_waits(nc, max_waits=_MAX_CTRL_WAITS):
    """This walrus build rejects >1 sem wait per compute instruction; hoist
    overflow waits onto same-engine NoOps placed just before (engine queues
    are strict FIFO, so the gating is equivalent)."""
    n_split = 0
    for f in nc.m.functions:
        for bb in f.blocks:
            new = []
            for inst in bb.instructions:
                si = inst.sync_info
                waits = list(si.on_wait) if si is not None and si.on_wait else []
                if len(waits) > max_waits:
                    extra = waits[: len(waits) - max_waits]
                    si.on_wait = waits[len(waits) - max_waits:]
                    for i in range(0, len(extra), max_waits):
                        nop = mybir.InstNoOp(
                            name=f"{inst.name}-xw{i}",
                            engine=inst.engine,
                            sync_info=mybir.SyncInfo(
                                on_wait=extra[i : i + max_waits], on_update=[]
                            ),
                            bass_nofuse=True,
                            ins=[],
                            outs=[],
                        )
                        new.append(nop)
                    n_split += 1
                new.append(inst)
            bb.instructions[:] = new
    return n_split


def build():
    _patch_tile_drain()
    nc = bass.Bass(trn_type="TRN2")

    AXW = 516  # per-f-chunk region width in the ax tiles (512 + shift fringe)
    XQW = T    # per-f-chunk width of the fp8 x copy (no halo; qkp reads
               # full token columns only). 4096 B stride: 16B-aligned for
               # the DoubleRow moving AP.

    xT = nc.dram_tensor("xT", [F, T], BF16, kind="ExternalInput")
    xQ = nc.dram_tensor("xQ", [128, NB * FC * TB], F8, kind="ExternalInput")
    wq = nc.dram_tensor("wq", [128, FC * H], F8, kind="ExternalInput")
    wk = nc.dram_tensor("wk", [128, FC * H], F8, kind="ExternalInput")
    cw = nc.dram_tensor("cw", [128, W * FC * H], BF16, kind="ExternalInput")
    vT = nc.dram_tensor("vT", [128, HC], BF16, kind="ExternalInput")
    ident = nc.dram_tensor("ident", [128, 128], BF16, kind="ExternalInput")
    # [NB*128, 4*H] bf16; token (blk*512 + g*128 + p) lives at row
    # (blk*128 + p), cols [g*512, (g+1)*512). Host de-interleaves + adds b.
    out = nc.dram_tensor("out", [NB * 128, 4 * H], BF16, kind="ExternalOutput")

    with ExitStack() as ctx:
        tc = ctx.enter_context(tile.TileContext(nc))

        const = ctx.enter_context(tc.tile_pool(name="const", bufs=1))
        wq_sb = const.tile([128, FC * H], F8, tag="wq")
        wk_sb = const.tile([128, FC * H], F8, tag="wk")
        cw_sb = const.tile([128, W * FC * H], BF16, tag="cw")
        vT_sb = const.tile([128, HC], BF16, tag="vT")
        id_sb = const.tile([128, 128], BF16, tag="ident")
        xt_sb = const.tile([128, FC * XW], BF16, tag="xt")
        xq_sb = const.tile([128, NB * FC * TB], F8, tag="xq")
        kp_sb = [const.tile([128, XW], BF16, name=f"kp{hc}", tag=f"kp{hc}") for hc in range(HC)]

        def dma_xq(b0, b1):
            # fp8 x for the q/k projections, block-major layout so each
            # partition's slice is (b1-b0)*2KB contiguous
            nc.sync.dma_start(
                xq_sb[:, b0 * FC * TB: b1 * FC * TB],
                xQ[:, b0 * FC * TB: b1 * FC * TB],
            )

        def dma_xt(fc, seg, half=None):
            o0, o1 = (0, S) if half is None else (half * (S // 2), (half + 1) * (S // 2))
            nc.sync.dma_start(
                xt_sb[:, fc * XW + seg * SEGW + PAD + o0: fc * XW + seg * SEGW + PAD + o1],
                xT[fc * 128:(fc + 1) * 128, seg * S + o0: seg * S + o1],
            )

        # issue order matters: the first qkp matmuls need xq(b0..b3) + wq.
        dma_xq(0, 1)
        nc.scalar.dma_start(wq_sb[:], wq[:])
        dma_xq(1, 4)
        nc.scalar.dma_start(wk_sb[:], wk[:])
        dma_xq(4, NB)
        nc.sync.dma_start(vT_sb[:], vT[:])
        nc.sync.dma_start(id_sb[:], ident[:])
        nc.sync.dma_start(cw_sb[:], cw[:])
        for fc in range(FC):
            dma_xt(fc, 0, 0)
            dma_xt(fc, 0, 1)
        for fc in range(FC):
            dma_xt(fc, 1)
        # zero the halo columns of x^T and kp^T
        for seg in range(BPC):
            for fc in range(FC):
                b0 = fc * XW + seg * SEGW
                nc.vector.memset(xt_sb[:, b0:b0 + PAD], 0.0)
                nc.vector.memset(xt_sb[:, b0 + PAD + S:b0 + SEGW], 0.0)
            for hc in range(HC):
                nc.vector.memset(kp_sb[hc][:, seg * SEGW: seg * SEGW + PAD], 0.0)
                nc.vector.memset(kp_sb[hc][:, seg * SEGW + PAD + S: (seg + 1) * SEGW], 0.0)

        # sc and tp share one pool/bank: scores -> exp -> transpose are strictly
        # sequential, freeing a bank so qkp gets 3
        qkp_ps = ctx.enter_context(tc.tile_pool(name="qkp_ps", bufs=4, space="PSUM"))
        sc_ps_pool = ctx.enter_context(tc.tile_pool(name="sc_ps", bufs=1, space="PSUM"))
        out_ps_pool = ctx.enter_context(tc.tile_pool(name="out_ps", bufs=3, space="PSUM"))

        qp = ctx.enter_context(tc.tile_pool(name="qp", bufs=2))
        argp = ctx.enter_context(tc.tile_pool(name="argp", bufs=2))
        thp = ctx.enter_context(tc.tile_pool(name="thp", bufs=5))
        smp = ctx.enter_context(tc.tile_pool(name="smp", bufs=4))
        attp = ctx.enter_context(tc.tile_pool(name="attp", bufs=2))
        flp = ctx.enter_context(tc.tile_pool(name="flp", bufs=2))
        bcp = ctx.enter_context(tc.tile_pool(name="bcp", bufs=2))
        axp = ctx.enter_context(tc.tile_pool(name="axp", bufs=7))
        outp = ctx.enter_context(tc.tile_pool(name="outp", bufs=2))

        def xcol0(b):
            return (b // NBSEG) * SEGW + PAD + (b % NBSEG) * TB

        q_tiles = {}
        th_tiles = {}
        attT_tiles = {}
        ab_tiles = {}
        ax_tiles = {}
        rc_tiles = {}
        ex_tiles = {}

        xq4 = xq_sb[:].rearrange("p (b f t) -> p b f t", b=NB, f=FC)
        wq3 = wq_sb[:].rearrange("p (f h) -> p f h", f=FC)
        wk3 = wk_sb[:].rearrange("p (f h) -> p f h", f=FC)

        def emit_qkp(b):
            qt = qp.tile([128, HC * TB], BF16, name="qt", tag="qt")
            q_tiles[b] = qt
            for hc in range(HC):
                for which, w3 in ((0, wq3), (1, wk3)):
                    ps = qkp_ps.tile([128, TB], F32, name="ps", tag="qkp")
                    for pi in range(2):
                        # fp8 DoubleRow: contraction 256 per matmul
                        nc.tensor.matmul(
                            ps[:],
                            w3[:, 2 * pi:2 * pi + 2, hc * 128:(hc + 1) * 128],
                            xq4[:, b, 2 * pi:2 * pi + 2, :],
                            start=(pi == 0),
                            stop=(pi == 1),
                            perf_mode=DR,
                        )
                    if which == 0:
                        nc.scalar.activation(qt[:, hc * TB:(hc + 1) * TB], ps[:], AF.Copy)
                    else:
                        nc.vector.tensor_copy(kp_sb[hc][:, xcol0(b):xcol0(b) + TB], ps[:])

        def emit_addstanh(b):
            xc = xcol0(b)
            qt = q_tiles.pop(b)
            ths = []
            for hc in range(HC):
                arg = argp.tile([128, W * TB], BF16, name="arg", tag="arg")
                for w in range(W):
                    nc.vector.tensor_add(
                        arg[:, w * TB:(w + 1) * TB],
                        kp_sb[hc][:, xc - PAD + w: xc - PAD + w + TB],
                        qt[:, hc * TB:(hc + 1) * TB],
                    )
                th = thp.tile([128, W * TB], BF16, name="th", tag="th")
                # q/kp carry a 64x fp8 weight pre-scale; descale inside tanh
                nc.scalar.activation(th[:], arg[:], AF.Tanh, scale=1.0 / WSCALE)
                ths.append(th)
            th_tiles[b] = ths

        def emit_scores(b):
            sc_ps = sc_ps_pool.tile([128, 32], F32, name="sc", tag="sc")
            ths = th_tiles.pop(b)
            for hc in range(HC):
                th = ths[hc]
                for w in range(W):
                    for g in range(4):
                        col = g * 8 + w
                        nc.tensor.matmul(
                            sc_ps[:, col:col + 1],
                            th[:, w * TB + g * 128: w * TB + (g + 1) * 128],
                            vT_sb[:, hc:hc + 1],
                            start=(hc == 0 and w == 0 and g == 0),
                            stop=(hc == HC - 1 and w == W - 1 and g == 3),
                            skip_group_check=True,
                        )
            # softmax over W=5 (scores bounded by |v|_1 ~ 8, no max-sub
            # needed). The broadcastable row tensor carries UNNORMALIZED
            # exp(scores); 1/sum folds into the cnn PSUM drain as a
            # per-partition ACT scale two iterations later, so the PE
            # transposes depend on nothing but the exp.
            attT = attp.tile([W, TB], BF16, name="attT", tag="attT")
            ex = smp.tile([128, 4 * W], BF16, name="ex", tag="ex")
            nc.scalar.activation(
                ex[:].rearrange("p (g w) -> p g w", g=4),
                sc_ps[:].rearrange("p (g w) -> p g w", g=4)[:, :, 0:W],
                AF.Exp,
            )
            tp = sc_ps_pool.tile([128, TB], BF16, name="tp", tag="sc")
            for g in range(4):
                nc.tensor.transpose(
                    tp[0:W, g * 128:(g + 1) * 128], ex[:, g * W:(g + 1) * W], id_sb[:]
                )
            nc.scalar.activation(attT[:], tp[0:W, :], AF.Copy)
            ex_tiles[b] = ex
            attT_tiles[b] = attT

        def emit_sumrecip(b):
            # deferred to the head of the following iteration: keeps DVE's
            # FIFO from idling at the iteration boundary (rc is only consumed
            # by the cnn drain two iterations later)
            ex = ex_tiles.pop(b)
            sm = smp.tile([128, 4], F32, name="sm", tag="sm")
            nc.vector.reduce_sum(sm[:], ex[:].rearrange("p (g w) -> p g w", g=4), AX.X)
            rc = smp.tile([128, 4], F32, name="rc", tag="rc")
            nc.vector.reciprocal(rc[:], sm[:])
            rc_tiles[b] = rc

        def emit_bcast(b):
            # two-hop att broadcast with fanned-out source reads:
            #   hop 1: attT row w -> cols [ABST*w, ABST*w+TB) of FOUR
            #          replica partitions (5 DMAs x 4 descriptors)
            #   hop 2: replica k -> dst partition quadrant k (4 DMAs x 32
            #          descriptors of 5.2KB, each quadrant reading its own
            #          source partition's port)
            # 148 descriptors/block instead of 640, without the
            # single-partition source port bottleneck of a 1->128 fan-out.
            attT = attT_tiles.pop(b)
            eng = nc.sync
            fl = flp.tile([64, FLW], BF16, name="fl", tag="fl")
            # hop 1a: flatten attT's 5 rows into replica row 0 (one dma,
            # 5 descriptors), then 1b: replicate row 0 to the other 7
            # port-spread rows. Two triggers instead of five.
            eng.dma_start(
                fl[0:1, :].rearrange("p (w j) -> p w j", w=W)[:, :, 0:TB],
                attT[0:W, :],
            )
            eng.dma_start(
                fl[8:64:8, 0:ABF],
                fl[0:1, 0:ABF].rearrange("p (r c) -> p r c", r=1)
                .broadcast_to((1, 7, ABF)),
            )
            # single broadcast whose 128 descriptors read from 8 source
            # partitions sitting on 8 different SBUF AXI ports (stride 8), so
            # the fan-out runs at ~8x the single-port read rate
            ab = bcp.tile([128, ABF], BF16, name="ab", tag="ab")
            eng.dma_start(
                ab[:],
                fl[0:64:8, 0:ABF].rearrange("p (r c) -> p r c", r=1)
                .broadcast_to((8, 16, ABF)),
            )
            ab_tiles[b] = ab

        def emit_ax(b):
            xc = xcol0(b)
            ab = ab_tiles.pop(b)
            axs = []
            for w in range(W):
                ax = axp.tile([128, FC * AXW], BF16, name="ax", tag="ax")
                # ax[:, fc, j] = xt[:, fc, xc-2+j] * att_w[j-w]   (j in [0,516))
                nc.vector.tensor_tensor(
                    ax.rearrange("p (f c) -> p f c", f=FC)[:, :, 0:516],
                    xt_sb.rearrange("p (f c) -> p f c", f=FC)[:, :, xc - PAD: xc - PAD + 516],
                    ab[:, ABW * w: ABW * w + 516].rearrange("p (r c) -> p r c", r=1)
                    .broadcast_to((128, FC, 516)),
                    mybir.AluOpType.mult,
                )
                axs.append(ax)
            ax_tiles[b] = axs

        def emit_cnnmm(b):
            axs = ax_tiles.pop(b)
            rc = rc_tiles.pop(b)
            ot = outp.tile([128, 4 * H], BF16, name="ot", tag="ot")
            # psum [token-group, h]; g-outer so each group's psum closes early
            # and the ACT drain (scaled by 1/softmax-sum per token partition)
            # spreads across the iteration instead of bunching at its end
            for g in range(4):
                op = out_ps_pool.tile([128, H], F32, name=f"op{g}", tag="op")
                for w in range(W):
                    ax = axs[w]
                    for fc in range(FC):
                        nc.tensor.matmul(
                            op[:],
                            ax[:, fc * AXW + w + g * 128: fc * AXW + w + (g + 1) * 128],
                            cw_sb[:, (w * FC + fc) * H:(w * FC + fc + 1) * H],
                            start=(w == 0 and fc == 0),
                            stop=(w == W - 1 and fc == FC - 1),
                        )
                nc.scalar.activation(ot[:, g * H:(g + 1) * H], op[:], AF.Copy, scale=rc[:, g:g + 1])
                if b == NB - 1 and g == 1:
                    # tail block: first half ships under the remaining
                    # matmuls; the last half goes via SWDGE whose
                    # completion tail is shorter than HWDGE's ~10us
                    nc.sync.dma_start(
                        out[b * 128:(b + 1) * 128, 0:2 * H], ot[:, 0:2 * H]
                    )
            if b < NB - 1:
                # one 4KB-per-partition store per block
                nc.sync.dma_start(out[b * 128:(b + 1) * 128, :], ot[:])
            else:
                nc.gpsimd.dma_start(
                    out[b * 128:(b + 1) * 128, 2 * H:4 * H], ot[:, 2 * H:4 * H]
                )

        # Software-pipelined emission. Per-engine FIFO program order within a
        # steady-state iteration:
        #   PE : qkp(b+4) | cnn-matmuls(b) | scores+transposes(b+2)
        #   DVE: args(b+2) | ax-mults(b+1) | softmax bits(b+2) | out-add(b)
        #   ACT: tanh(b+2) | q-copy(b+4) | exp/attT(b+2)
        # so by the time PE reaches scores(b+2) the tanh chain has had a full
        # cnn's worth of slack, and cnn(b+1)'s ax tiles were multiplied one
        # iteration ahead (broadcast DMA latency hidden behind cnn(b)).
        emit_qkp(0)
        emit_qkp(1)
        emit_addstanh(0)
        emit_qkp(2)
        emit_qkp(3)
        emit_scores(0)
        emit_bcast(0)
        emit_sumrecip(0)
        emit_addstanh(1)
        emit_scores(1)
        emit_bcast(1)
        emit_sumrecip(1)
        emit_addstanh(2)
        emit_ax(0)
        # steady state: scores(b+2) run at the START of iter b (their tanh ran
        # last iter), so attT/bcast fire ~20us before ax(b+2) needs them and
        # the attention chain never gates the PE
        for b in range(NB):
            if b + 4 < NB:
                emit_qkp(b + 4)
            if b + 2 < NB:
                emit_scores(b + 2)
                emit_bcast(b + 2)
                emit_sumrecip(b + 2)
            if b + 3 < NB:
                emit_addstanh(b + 3)
            if b + 1 < NB:
                emit_ax(b + 1)
            emit_cnnmm(b)

    _split_multi_waits(nc)
    return nc


_NC_CACHE = None


def _get_nc():
    global _NC_CACHE
    if _NC_CACHE is None:
        _NC_CACHE = build()
    return _NC_CACHE


def _prep_in_maps(embeds_output, Wq, Wk, v_att, cnn_W):
    bf = ml_dtypes.bfloat16
    f8 = ml_dtypes.float8_e4m3
    # wq/wk: fp8, pre-scaled by WSCALE, packed [128, FC*H]
    # (partition p holds concat over fc of W[fc*128+p, :])
    wqs = np.asarray(Wq, dtype=np.float32) * WSCALE
    wks = np.asarray(Wk, dtype=np.float32) * WSCALE
    wq8 = np.ascontiguousarray(
        wqs.reshape(FC, 128, H).transpose(1, 0, 2).reshape(128, FC * H)
    ).astype(f8)
    wk8 = np.ascontiguousarray(
        wks.reshape(FC, 128, H).transpose(1, 0, 2).reshape(128, FC * H)
    ).astype(f8)
    # cnn_W: bf16, packed [128, W*FC*H]
    cwf = np.asarray(cnn_W, dtype=np.float32)
    cwp = np.ascontiguousarray(
        cwf.reshape(W * FC, 128, H).transpose(1, 0, 2).reshape(128, W * FC * H)
    ).astype(bf)
    vTp = np.ascontiguousarray(
        np.asarray(v_att, dtype=np.float32).reshape(HC, 128).T
    ).astype(bf)
    identp = np.eye(128, dtype=np.float32).astype(bf)

    x = np.asarray(embeds_output, dtype=np.float32)
    in_maps = []
    for c in range(NCORES):
        shard = x[c * BPC:(c + 1) * BPC]                  # [BPC, S, F]
        xTs = shard.transpose(2, 0, 1).reshape(F, T)      # [F, BPC*S]
        xTb = np.ascontiguousarray(xTs).astype(bf)
        # fp8 copy in [128, FC*T] layout, quantized from the bf16 values
        # the device sees (matches the accuracy sim)
        xq8 = np.ascontiguousarray(
            xTb.astype(np.float32).reshape(FC, 128, NB, TB)
            .transpose(1, 2, 0, 3).reshape(128, NB * FC * TB)
        ).astype(f8)
        in_maps.append(
            {
                "xT": xTb,
                "xQ": xq8,
                "wq": wq8,
                "wk": wk8,
                "cw": cwp,
                "vT": vTp,
                "ident": identp,
            }
        )
    return in_maps


def kernel(embeds_output, Wq, Wk, v_att, cnn_W, cnn_b, **run_kwargs):
    nc = _get_nc()
    in_maps = _prep_in_maps(embeds_output, Wq, Wk, v_att, cnn_W)
    res = run_bass_kernel_spmd(nc, in_maps, core_ids=list(range(NCORES)), **run_kwargs)
    # device output is [NB*128, 4*H] bf16 per core: token blk*512+g*128+p at
    # (blk*128+p, g*512:+512). De-interleave + upcast + add bias here.
    shards = []
    for c in range(NCORES):
        o = np.asarray(res.results[c]["out"], dtype=np.float32)
        o = o.reshape(NB, 128, 4, H).transpose(0, 2, 1, 3).reshape(BPC, S, H)
        shards.append(o)
    full = np.concatenate(shards, axis=0) + np.asarray(cnn_b, dtype=np.float32)
    kernel.last_results = res
    return full


# revision 45
# speedup vs baseline: 1.0014x; 1.0014x over previous
"""ACNN sparse-attention Trainium2 kernel (8 NeuronCores, pure data parallel).

Reference computation (per batch b, token s):
  windows[s, w]   = x[s + w - 2]           (zero-padded outside the sequence)
  q               = x[s] @ Wq
  kp[s']          = x[s'] @ Wk             (zero rows stay zero)
  scores[s, w]    = sum_h v[h] * tanh(kp[s + w - 2, h] + q[s, h])
  att             = softmax_w(scores)
  out[s]          = sum_w att[s, w] * (x[s + w - 2] @ cnn_W[w*F:(w+1)*F]) + b

Sharding: batch 16 -> 2 per core. Everything on-device is computed from
x^T ([F, tokens]) so all matmul contractions sit on the partition dim;
the W=5 window shifts become free-dim column shifts of x^T / kp^T.

v2 changes vs baseline (297us -> ~227us):
  - q/k projections run as fp8e4 DoubleRow matmuls (x and Wq/Wk quantized
    to e4m3, weights pre-scaled by 64; the 1/64 descale folds into the
    tanh's ACT input scale). Halves the qkp PE stream time (55us -> 29us).
  - the att broadcast (the baseline's hidden co-bottleneck: all of its
    fan-out DMAs read SBUF partitions 0-4, which share one 27 GB/s AXI
    port, ~24us/block serialized) is now a 2-hop spread-source broadcast:
    attT's 5 rows are first flattened into 8 replica rows at partition
    stride 8 (8 different AXI ports), then ONE dma fans [8,16,2596] out
    to all 128 partitions reading 8 ports in parallel (~3us/block, 168
    descriptors instead of 640).
  - out stores go out as one [128, 4*H] DMA per block (4KB/partition
    descriptors); the host de-interleaves the (block, partition, group)
    token order. The tail block stores per-group so the last store
    overlaps the final matmuls.
  - weights (wq/wk/cw) and the fp8 x copy are host-prepacked into their
    SBUF layouts so each loads with fat contiguous-per-partition
    descriptors, and the in-loop DMAs (broadcast chain + out) no longer
    share a queue with the input stream (no head-of-line blocking).
"""
import sys

sys.path.insert(0, "/opt/trn_rl_repo")

from contextlib import ExitStack

import numpy as np
import ml_dtypes

import concourse.bass as bass
import concourse.tile as tile
from concourse import mybir
from concourse.bass_utils import run_bass_kernel_spmd

def _install_ntff_hook_shim():
    """The image's antenv package lacks axon_hooks; recreate it so
    run_bass_kernel_spmd(trace=True) can capture NTFF profiles."""
    import types

    if "antenv.axon_hooks" in sys.modules:
        return
    mod = types.ModuleType("antenv.axon_hooks")
    mod._hook = None
    mod.set_axon_ntff_profile_hook = lambda h: setattr(mod, "_hook", h)
    mod.get_axon_ntff_profile_hook = lambda: mod._hook
    sys.modules["antenv.axon_hooks"] = mod
    try:
        sys.path.insert(0, "/root/.axon_site/trn_agent_boot")
        import trn_boot

        hook = trn_boot._ntff_profile_via_ctypes("/opt/axon/libaxon_pjrt.so")
        if hook is not None:
            mod._hook = hook
    except Exception:
        pass


_install_ntff_hook_shim()

F32 = mybir.dt.float32
BF16 = mybir.dt.bfloat16
F8 = mybir.dt.float8e4
AF = mybir.ActivationFunctionType
AX = mybir.AxisListType
DR = mybir.MatmulPerfMode.DoubleRow

B, S, F, H, W = 16, 2048, 512, 512, 5
PAD = W // 2
NCORES = 8
BPC = B // NCORES          # batches per core
T = BPC * S                # tokens per core (4096)
TB = 512                   # token block
NB = T // TB               # 8 blocks per core
NBSEG = S // TB            # 4 blocks per segment (batch)
FC = F // 128              # 4 f-chunks
HC = H // 128              # 4 h-chunks
SEGW = S + 2 * PAD         # padded segment width in SBUF (2052)
XW = BPC * SEGW            # padded token width in SBUF (4104)
WSCALE = 64.0              # fp8 pre-scale on Wq/Wk (descale folds into tanh)

ABW = 520                  # per-w read-window stride in the bcast ab tile (even)
ABST = ABW + 1             # att_w starts at col ABST*w (offset w inside window)
ABF = (W - 1) * ABST + TB  # used width of the flat/ab rows (2596)
FLW = W * ABST             # allocated flat row width (2605)

_MAX_CTRL_WAITS = 1


def _patch_tile_drain():
    """walrus rejects >4 sem waits on one CTRL; spread the TileContext exit
    drain's waits over extra drain instructions."""
    if getattr(tile.TileContext, "_acnn_drain_patched", False):
        return
    from concourse.vector_clock import ScopedClock

    def _drain_and_barrier(self, tick_clock, wait_clock):
        DRAIN_WAITS = 4  # CTRL instructions accept up to 4 sem waits
        nc = self.nc
        drain_inst = nc.sync.drain()
        wait_clock.add_sem_waits(
            drain_inst.ins, ScopedClock({None: tick_clock.global_clock})
        )
        si = drain_inst.ins.sync_info
        waits = list(si.on_wait) if si and si.on_wait else []
        if len(waits) > DRAIN_WAITS:
            si.on_wait = waits[:DRAIN_WAITS]
            rest = waits[DRAIN_WAITS:]
            for i in range(0, len(rest), DRAIN_WAITS):
                extra = nc.sync.drain()
                esi = extra.ins.sync_info
                chunk = rest[i : i + DRAIN_WAITS]
                if esi is None:
                    extra.ins.sync_info = type(si)(on_wait=chunk, on_update=[])
                else:
                    esi.on_wait = chunk
        nc.all_engine_barrier()
        popped = nc._tile_sem_poison_stack.pop()
        assert popped is self._sem_poison
        nc.clear_and_free_semaphores(list(self.sems.allocated().values()))
        nc.all_engine_barrier()

    tile.TileContext._drain_and_barrier = _drain_and_barrier
    tile.TileContext._acnn_drain_patched = True


def _split_multi_waits(nc, max_waits=_MAX_CTRL_WAITS):
    """This walrus build rejects >1 sem wait per compute instruction; hoist
    overflow waits onto same-engine NoOps placed just before (engine queues
    are strict FIFO, so the gating is equivalent)."""
    n_split = 0
    for f in nc.m.functions:
        for bb in f.blocks:
            new = []
            for inst in bb.instructions:
                si = inst.sync_info
                waits = list(si.on_wait) if si is not None and si.on_wait else []
                if len(waits) > max_waits:
                    extra = waits[: len(waits) - max_waits]
                    si.on_wait = waits[len(waits) - max_waits:]
                    for i in range(0, len(extra), max_waits):
                        nop = mybir.InstNoOp(
                            name=f"{inst.name}-xw{i}",
                            engine=inst.engine,
                            sync_info=mybir.SyncInfo(
                                on_wait=extra[i : i + max_waits], on_update=[]
                            ),
                            bass_nofuse=True,
                            ins=[],
                            outs=[],
                        )
                        new.append(nop)
                    n_split += 1
                new.append(inst)
            bb.instructions[:] = new
    return n_split


def build():
    _patch_tile_drain()
    nc = bass.Bass(trn_type="TRN2")

    AXW = 516  # per-f-chunk region width in the ax tiles (512 + shift fringe)
    XQW = T    # per-f-chunk width of the fp8 x copy (no halo; qkp reads
               # full token columns only). 4096 B stride: 16B-aligned for
               # the DoubleRow moving AP.

    xT = nc.dram_tensor("xT", [F, T], BF16, kind="ExternalInput")
    xQ = nc.dram_tensor("xQ", [128, NB * FC * TB], F8, kind="ExternalInput")
    wq = nc.dram_tensor("wq", [128, FC * H], F8, kind="ExternalInput")
    wk = nc.dram_tensor("wk", [128, FC * H], F8, kind="ExternalInput")
    cw = nc.dram_tensor("cw", [128, W * FC * H], BF16, kind="ExternalInput")
    vT = nc.dram_tensor("vT", [128, HC], BF16, kind="ExternalInput")
    ident = nc.dram_tensor("ident", [128, 128], BF16, kind="ExternalInput")
    # [NB*128, 4*H] bf16; token (blk*512 + g*128 + p) lives at row
    # (blk*128 + p), cols [g*512, (g+1)*512). Host de-interleaves + adds b.
    out = nc.dram_tensor("out", [NB * 128, 4 * H], BF16, kind="ExternalOutput")

    with ExitStack() as ctx:
        tc = ctx.enter_context(tile.TileContext(nc))

        const = ctx.enter_context(tc.tile_pool(name="const", bufs=1))
        wq_sb = const.tile([128, FC * H], F8, tag="wq")
        wk_sb = const.tile([128, FC * H], F8, tag="wk")
        cw_sb = const.tile([128, W * FC * H], BF16, tag="cw")
        vT_sb = const.tile([128, HC], BF16, tag="vT")
        id_sb = const.tile([128, 128], BF16, tag="ident")
        xt_sb = const.tile([128, FC * XW], BF16, tag="xt")
        xq_sb = const.tile([128, NB * FC * TB], F8, tag="xq")
        kp_sb = [const.tile([128, XW], BF16, name=f"kp{hc}", tag=f"kp{hc}") for hc in range(HC)]

        def dma_xq(b0, b1):
            # fp8 x for the q/k projections, block-major layout so each
            # partition's slice is (b1-b0)*2KB contiguous
            nc.sync.dma_start(
                xq_sb[:, b0 * FC * TB: b1 * FC * TB],
                xQ[:, b0 * FC * TB: b1 * FC * TB],
            )

        def dma_xt(fc, seg, half=None):
            o0, o1 = (0, S) if half is None else (half * (S // 2), (half + 1) * (S // 2))
            nc.sync.dma_start(
                xt_sb[:, fc * XW + seg * SEGW + PAD + o0: fc * XW + seg * SEGW + PAD + o1],
                xT[fc * 128:(fc + 1) * 128, seg * S + o0: seg * S + o1],
            )

        # issue order matters: the first qkp matmuls need xq(b0..b3) + wq.
        dma_xq(0, 1)
        nc.scalar.dma_start(wq_sb[:], wq[:])
        dma_xq(1, 4)
        nc.scalar.dma_start(wk_sb[:], wk[:])
        dma_xq(4, NB)
        nc.sync.dma_start(vT_sb[:], vT[:])
        nc.sync.dma_start(id_sb[:], ident[:])
        nc.sync.dma_start(cw_sb[:], cw[:])
        for fc in range(FC):
            dma_xt(fc, 0, 0)
            dma_xt(fc, 0, 1)
        for fc in range(FC):
            dma_xt(fc, 1)
        # zero the halo columns of x^T and kp^T
        for seg in range(BPC):
            for fc in range(FC):
                b0 = fc * XW + seg * SEGW
                nc.vector.memset(xt_sb[:, b0:b0 + PAD], 0.0)
                nc.vector.memset(xt_sb[:, b0 + PAD + S:b0 + SEGW], 0.0)
            for hc in range(HC):
                nc.vector.memset(kp_sb[hc][:, seg * SEGW: seg * SEGW + PAD], 0.0)
                nc.vector.memset(kp_sb[hc][:, seg * SEGW + PAD + S: (seg + 1) * SEGW], 0.0)

        # sc and tp share one pool/bank: scores -> exp -> transpose are strictly
        # sequential, freeing a bank so qkp gets 3
        qkp_ps = ctx.enter_context(tc.tile_pool(name="qkp_ps", bufs=4, space="PSUM"))
        sc_ps_pool = ctx.enter_context(tc.tile_pool(name="sc_ps", bufs=1, space="PSUM"))
        out_ps_pool = ctx.enter_context(tc.tile_pool(name="out_ps", bufs=3, space="PSUM"))

        qp = ctx.enter_context(tc.tile_pool(name="qp", bufs=2))
        argp = ctx.enter_context(tc.tile_pool(name="argp", bufs=2))
        thp = ctx.enter_context(tc.tile_pool(name="thp", bufs=5))
        smp = ctx.enter_context(tc.tile_pool(name="smp", bufs=4))
        attp = ctx.enter_context(tc.tile_pool(name="attp", bufs=2))
        flp = ctx.enter_context(tc.tile_pool(name="flp", bufs=2))
        bcp = ctx.enter_context(tc.tile_pool(name="bcp", bufs=2))
        axp = ctx.enter_context(tc.tile_pool(name="axp", bufs=7))
        outp = ctx.enter_context(tc.tile_pool(name="outp", bufs=2))

        def xcol0(b):
            return (b // NBSEG) * SEGW + PAD + (b % NBSEG) * TB

        q_tiles = {}
        th_tiles = {}
        attT_tiles = {}
        ab_tiles = {}
        ax_tiles = {}
        rc_tiles = {}
        ex_tiles = {}

        xq4 = xq_sb[:].rearrange("p (b f t) -> p b f t", b=NB, f=FC)
        wq3 = wq_sb[:].rearrange("p (f h) -> p f h", f=FC)
        wk3 = wk_sb[:].rearrange("p (f h) -> p f h", f=FC)

        def emit_qkp(b):
            qt = qp.tile([128, HC * TB], BF16, name="qt", tag="qt")
            q_tiles[b] = qt
            for hc in range(HC):
                for which, w3 in ((0, wq3), (1, wk3)):
                    ps = qkp_ps.tile([128, TB], F32, name="ps", tag="qkp")
                    for pi in range(2):
                        # fp8 DoubleRow: contraction 256 per matmul
                        nc.tensor.matmul(
                            ps[:],
                            w3[:, 2 * pi:2 * pi + 2, hc * 128:(hc + 1) * 128],
                            xq4[:, b, 2 * pi:2 * pi + 2, :],
                            start=(pi == 0),
                            stop=(pi == 1),
                            perf_mode=DR,
                        )
                    if which == 0:
                        nc.scalar.activation(qt[:, hc * TB:(hc + 1) * TB], ps[:], AF.Copy)
                    else:
                        nc.vector.tensor_copy(kp_sb[hc][:, xcol0(b):xcol0(b) + TB], ps[:])

        def emit_addstanh(b):
            xc = xcol0(b)
            qt = q_tiles.pop(b)
            ths = []
            for hc in range(HC):
                arg = argp.tile([128, W * TB], BF16, name="arg", tag="arg")
                for w in range(W):
                    nc.vector.tensor_add(
                        arg[:, w * TB:(w + 1) * TB],
                        kp_sb[hc][:, xc - PAD + w: xc - PAD + w + TB],
                        qt[:, hc * TB:(hc + 1) * TB],
                    )
                th = thp.tile([128, W * TB], BF16, name="th", tag="th")
                # q/kp carry a 64x fp8 weight pre-scale; descale inside tanh
                nc.scalar.activation(th[:], arg[:], AF.Tanh, scale=1.0 / WSCALE)
                ths.append(th)
            th_tiles[b] = ths

        def emit_scores(b):
            sc_ps = sc_ps_pool.tile([128, 32], F32, name="sc", tag="sc")
            ths = th_tiles.pop(b)
            for hc in range(HC):
                th = ths[hc]
                for w in range(W):
                    for g in range(4):
                        col = g * 8 + w
                        nc.tensor.matmul(
                            sc_ps[:, col:col + 1],
                            th[:, w * TB + g * 128: w * TB + (g + 1) * 128],
                            vT_sb[:, hc:hc + 1],
                            start=(hc == 0 and w == 0 and g == 0),
                            stop=(hc == HC - 1 and w == W - 1 and g == 3),
                            skip_group_check=True,
                        )
            # softmax over W=5 (scores bounded by |v|_1 ~ 8, no max-sub
            # needed). The broadcastable row tensor carries UNNORMALIZED
            # exp(scores); 1/sum folds into the cnn PSUM drain as a
            # per-partition ACT scale two iterations later, so the PE
            # transposes depend on nothing but the exp.
            attT = attp.tile([W, TB], BF16, name="attT", tag="attT")
            ex = smp.tile([128, 4 * W], BF16, name="ex", tag="ex")
            nc.scalar.activation(
                ex[:].rearrange("p (g w) -> p g w", g=4),
                sc_ps[:].rearrange("p (g w) -> p g w", g=4)[:, :, 0:W],
                AF.Exp,
            )
            tp = sc_ps_pool.tile([128, TB], BF16, name="tp", tag="sc")
            for g in range(4):
                nc.tensor.transpose(
                    tp[0:W, g * 128:(g + 1) * 128], ex[:, g * W:(g + 1) * W], id_sb[:]
                )
            nc.scalar.activation(attT[:], tp[0:W, :], AF.Copy)
            ex_tiles[b] = ex
            attT_tiles[b] = attT

        def emit_sumrecip(b):
            # deferred to the head of the following iteration: keeps DVE's
            # FIFO from idling at the iteration boundary (rc is only consumed
            # by the cnn drain two iterations later)
            ex = ex_tiles.pop(b)
            sm = smp.tile([128, 4], F32, name="sm", tag="sm")
            nc.vector.reduce_sum(sm[:], ex[:].rearrange("p (g w) -> p g w", g=4), AX.X)
            rc = smp.tile([128, 4], F32, name="rc", tag="rc")
            nc.vector.reciprocal(rc[:], sm[:])
            rc_tiles[b] = rc

        def emit_bcast(b):
            # two-hop att broadcast with fanned-out source reads:
            #   hop 1: attT row w -> cols [ABST*w, ABST*w+TB) of FOUR
            #          replica partitions (5 DMAs x 4 descriptors)
            #   hop 2: replica k -> dst partition quadrant k (4 DMAs x 32
            #          descriptors of 5.2KB, each quadrant reading its own
            #          source partition's port)
            # 148 descriptors/block instead of 640, without the
            # single-partition source port bottleneck of a 1->128 fan-out.
            attT = attT_tiles.pop(b)
            eng = nc.sync
            fl = flp.tile([64, FLW], BF16, name="fl", tag="fl")
            # hop 1a: flatten attT's 5 rows into replica row 0 (one dma,
            # 5 descriptors), then 1b: replicate row 0 to the other 7
            # port-spread rows. Two triggers instead of five.
            eng.dma_start(
                fl[0:1, :].rearrange("p (w j) -> p w j", w=W)[:, :, 0:TB],
                attT[0:W, :],
            )
            eng.dma_start(
                fl[8:64:8, 0:ABF],
                fl[0:1, 0:ABF].rearrange("p (r c) -> p r c", r=1)
                .broadcast_to((1, 7, ABF)),
            )
            # single broadcast whose 128 descriptors read from 8 source
            # partitions sitting on 8 different SBUF AXI ports (stride 8), so
            # the fan-out runs at ~8x the single-port read rate
            ab = bcp.tile([128, ABF], BF16, name="ab", tag="ab")
            eng.dma_start(
                ab[:],
                fl[0:64:8, 0:ABF].rearrange("p (r c) -> p r c", r=1)
                .broadcast_to((8, 16, ABF)),
            )
            ab_tiles[b] = ab

        def emit_ax(b):
            xc = xcol0(b)
            ab = ab_tiles.pop(b)
            axs = []
            for w in range(W):
                ax = axp.tile([128, FC * AXW], BF16, name="ax", tag="ax")
                # ax[:, fc, j] = xt[:, fc, xc-2+j] * att_w[j-w]   (j in [0,516))
                nc.vector.tensor_tensor(
                    ax.rearrange("p (f c) -> p f c", f=FC)[:, :, 0:516],
                    xt_sb.rearrange("p (f c) -> p f c", f=FC)[:, :, xc - PAD: xc - PAD + 516],
                    ab[:, ABW * w: ABW * w + 516].rearrange("p (r c) -> p r c", r=1)
                    .broadcast_to((128, FC, 516)),
                    mybir.AluOpType.mult,
                )
                axs.append(ax)
            ax_tiles[b] = axs

        def emit_cnnmm(b):
            axs = ax_tiles.pop(b)
            rc = rc_tiles.pop(b)
            ot = outp.tile([128, 4 * H], BF16, name="ot", tag="ot")
            # psum [token-group, h]; g-outer so each group's psum closes early
            # and the ACT drain (scaled by 1/softmax-sum per token partition)
            # spreads across the iteration instead of bunching at its end
            for g in range(4):
                op = out_ps_pool.tile([128, H], F32, name=f"op{g}", tag="op")
                for w in range(W):
                    ax = axs[w]
                    for fc in range(FC):
                        nc.tensor.matmul(
                            op[:],
                            ax[:, fc * AXW + w + g * 128: fc * AXW + w + (g + 1) * 128],
                            cw_sb[:, (w * FC + fc) * H:(w * FC + fc + 1) * H],
                            start=(w == 0 and fc == 0),
                            stop=(w == W - 1 and fc == FC - 1),
                        )
                nc.scalar.activation(ot[:, g * H:(g + 1) * H], op[:], AF.Copy, scale=rc[:, g:g + 1])
            # one 4KB-per-partition store per block; tail-store variants
            # (per-group early stores, SWDGE final half) all measure within
            # noise -- the final store's latency hides under the exit drain
            nc.sync.dma_start(out[b * 128:(b + 1) * 128, :], ot[:])

        # Software-pipelined emission. Per-engine FIFO program order within a
        # steady-state iteration:
        #   PE : qkp(b+4) | cnn-matmuls(b) | scores+transposes(b+2)
        #   DVE: args(b+2) | ax-mults(b+1) | softmax bits(b+2) | out-add(b)
        #   ACT: tanh(b+2) | q-copy(b+4) | exp/attT(b+2)
        # so by the time PE reaches scores(b+2) the tanh chain has had a full
        # cnn's worth of slack, and cnn(b+1)'s ax tiles were multiplied one
        # iteration ahead (broadcast DMA latency hidden behind cnn(b)).
        emit_qkp(0)
        emit_qkp(1)
        emit_addstanh(0)
        emit_qkp(2)
        emit_qkp(3)
        emit_scores(0)
        emit_bcast(0)
        emit_sumrecip(0)
        emit_addstanh(1)
        emit_scores(1)
        emit_bcast(1)
        emit_sumrecip(1)
        emit_addstanh(2)
        emit_ax(0)
        # steady state: scores(b+2) run at the START of iter b (their tanh ran
        # last iter), so attT/bcast fire ~20us before ax(b+2) needs them and
        # the attention chain never gates the PE
        for b in range(NB):
            if b + 4 < NB:
                emit_qkp(b + 4)
            if b + 2 < NB:
                emit_scores(b + 2)
                emit_bcast(b + 2)
                emit_sumrecip(b + 2)
            if b + 3 < NB:
                emit_addstanh(b + 3)
            if b + 1 < NB:
                emit_ax(b + 1)
            emit_cnnmm(b)

    _split_multi_waits(nc)
    return nc


_NC_CACHE = None


def _get_nc():
    global _NC_CACHE
    if _NC_CACHE is None:
        _NC_CACHE = build()
    return _NC_CACHE


def _prep_in_maps(embeds_output, Wq, Wk, v_att, cnn_W):
    bf = ml_dtypes.bfloat16
    f8 = ml_dtypes.float8_e4m3
    # wq/wk: fp8, pre-scaled by WSCALE, packed [128, FC*H]
    # (partition p holds concat over fc of W[fc*128+p, :])
    wqs = np.asarray(Wq, dtype=np.float32) * WSCALE
    wks = np.asarray(Wk, dtype=np.float32) * WSCALE
    wq8 = np.ascontiguousarray(
        wqs.reshape(FC, 128, H).transpose(1, 0, 2).reshape(128, FC * H)
    ).astype(f8)
    wk8 = np.ascontiguousarray(
        wks.reshape(FC, 128, H).transpose(1, 0, 2).reshape(128, FC * H)
    ).astype(f8)
    # cnn_W: bf16, packed [128, W*FC*H]
    cwf = np.asarray(cnn_W, dtype=np.float32)
    cwp = np.ascontiguousarray(
        cwf.reshape(W * FC, 128, H).transpose(1, 0, 2).reshape(128, W * FC * H)
    ).astype(bf)
    vTp = np.ascontiguousarray(
        np.asarray(v_att, dtype=np.float32).reshape(HC, 128).T
    ).astype(bf)
    identp = np.eye(128, dtype=np.float32).astype(bf)

    x = np.asarray(embeds_output, dtype=np.float32)
    in_maps = []
    for c in range(NCORES):
        shard = x[c * BPC:(c + 1) * BPC]                  # [BPC, S, F]
        xTs = shard.transpose(2, 0, 1).reshape(F, T)      # [F, BPC*S]
        xTb = np.ascontiguousarray(xTs).astype(bf)
        # fp8 copy in [128, FC*T] layout, quantized from the bf16 values
        # the device sees (matches the accuracy sim)
        xq8 = np.ascontiguousarray(
            xTb.astype(np.float32).reshape(FC, 128, NB, TB)
            .transpose(1, 2, 0, 3).reshape(128, NB * FC * TB)
        ).astype(f8)
        in_maps.append(
            {
                "xT": xTb,
                "xQ": xq8,
                "wq": wq8,
                "wk": wk8,
                "cw": cwp,
                "vT": vTp,
                "ident": identp,
            }
        )
    return in_maps


def kernel(embeds_output, Wq, Wk, v_att, cnn_W, cnn_b, **run_kwargs):
    nc = _get_nc()
    in_maps = _prep_in_maps(embeds_output, Wq, Wk, v_att, cnn_W)
    res = run_bass_kernel_spmd(nc, in_maps, core_ids=list(range(NCORES)), **run_kwargs)
    # device output is [NB*128, 4*H] bf16 per core: token blk*512+g*128+p at
    # (blk*128+p, g*512:+512). De-interleave + upcast + add bias here.
    shards = []
    for c in range(NCORES):
        o = np.asarray(res.results[c]["out"], dtype=np.float32)
        o = o.reshape(NB, 128, 4, H).transpose(0, 2, 1, 3).reshape(BPC, S, H)
        shards.append(o)
    full = np.concatenate(shards, axis=0) + np.asarray(cnn_b, dtype=np.float32)
    kernel.last_results = res
    return full


# revision 46
# speedup vs baseline: 1.0131x; 1.0116x over previous
"""ACNN sparse-attention Trainium2 kernel (8 NeuronCores, pure data parallel).

Reference computation (per batch b, token s):
  windows[s, w]   = x[s + w - 2]           (zero-padded outside the sequence)
  q               = x[s] @ Wq
  kp[s']          = x[s'] @ Wk             (zero rows stay zero)
  scores[s, w]    = sum_h v[h] * tanh(kp[s + w - 2, h] + q[s, h])
  att             = softmax_w(scores)
  out[s]          = sum_w att[s, w] * (x[s + w - 2] @ cnn_W[w*F:(w+1)*F]) + b

Sharding: batch 16 -> 2 per core. Everything on-device is computed from
x^T ([F, tokens]) so all matmul contractions sit on the partition dim;
the W=5 window shifts become free-dim column shifts of x^T / kp^T.

v2 changes vs baseline (297us -> ~227us):
  - q/k projections run as fp8e4 DoubleRow matmuls (x and Wq/Wk quantized
    to e4m3, weights pre-scaled by 64; the 1/64 descale folds into the
    tanh's ACT input scale). Halves the qkp PE stream time (55us -> 29us).
  - the att broadcast (the baseline's hidden co-bottleneck: all of its
    fan-out DMAs read SBUF partitions 0-4, which share one 27 GB/s AXI
    port, ~24us/block serialized) is now a 2-hop spread-source broadcast:
    attT's 5 rows are first flattened into 8 replica rows at partition
    stride 8 (8 different AXI ports), then ONE dma fans [8,16,2596] out
    to all 128 partitions reading 8 ports in parallel (~3us/block, 168
    descriptors instead of 640).
  - out stores go out as one [128, 4*H] DMA per block (4KB/partition
    descriptors); the host de-interleaves the (block, partition, group)
    token order. The tail block stores per-group so the last store
    overlaps the final matmuls.
  - weights (wq/wk/cw) and the fp8 x copy are host-prepacked into their
    SBUF layouts so each loads with fat contiguous-per-partition
    descriptors, and the in-loop DMAs (broadcast chain + out) no longer
    share a queue with the input stream (no head-of-line blocking).
"""
import sys

sys.path.insert(0, "/opt/trn_rl_repo")

from contextlib import ExitStack

import numpy as np
import ml_dtypes

import concourse.bass as bass
import concourse.tile as tile
from concourse import mybir
from concourse.bass_utils import run_bass_kernel_spmd

def _install_ntff_hook_shim():
    """The image's antenv package lacks axon_hooks; recreate it so
    run_bass_kernel_spmd(trace=True) can capture NTFF profiles."""
    import types

    if "antenv.axon_hooks" in sys.modules:
        return
    mod = types.ModuleType("antenv.axon_hooks")
    mod._hook = None
    mod.set_axon_ntff_profile_hook = lambda h: setattr(mod, "_hook", h)
    mod.get_axon_ntff_profile_hook = lambda: mod._hook
    sys.modules["antenv.axon_hooks"] = mod
    try:
        sys.path.insert(0, "/root/.axon_site/trn_agent_boot")
        import trn_boot

        hook = trn_boot._ntff_profile_via_ctypes("/opt/axon/libaxon_pjrt.so")
        if hook is not None:
            mod._hook = hook
    except Exception:
        pass


_install_ntff_hook_shim()

F32 = mybir.dt.float32
BF16 = mybir.dt.bfloat16
F8 = mybir.dt.float8e4
AF = mybir.ActivationFunctionType
AX = mybir.AxisListType
DR = mybir.MatmulPerfMode.DoubleRow

B, S, F, H, W = 16, 2048, 512, 512, 5
PAD = W // 2
NCORES = 8
BPC = B // NCORES          # batches per core
T = BPC * S                # tokens per core (4096)
TB = 512                   # token block
NB = T // TB               # 8 blocks per core
NBSEG = S // TB            # 4 blocks per segment (batch)
FC = F // 128              # 4 f-chunks
HC = H // 128              # 4 h-chunks
SEGW = S + 2 * PAD         # padded segment width in SBUF (2052)
XW = BPC * SEGW            # padded token width in SBUF (4104)
WSCALE = 64.0              # fp8 pre-scale on Wq/Wk (descale folds into tanh)

ABW = 520                  # per-w read-window stride in the bcast ab tile (even)
ABST = ABW + 1             # att_w starts at col ABST*w (offset w inside window)
ABF = (W - 1) * ABST + TB  # used width of the flat/ab rows (2596)
FLW = W * ABST             # allocated flat row width (2605)

_MAX_CTRL_WAITS = 1


def _patch_tile_drain():
    """walrus rejects >4 sem waits on one CTRL; spread the TileContext exit
    drain's waits over extra drain instructions."""
    if getattr(tile.TileContext, "_acnn_drain_patched", False):
        return
    from concourse.vector_clock import ScopedClock

    def _drain_and_barrier(self, tick_clock, wait_clock):
        DRAIN_WAITS = 4  # CTRL instructions accept up to 4 sem waits
        nc = self.nc
        drain_inst = nc.sync.drain()
        wait_clock.add_sem_waits(
            drain_inst.ins, ScopedClock({None: tick_clock.global_clock})
        )
        si = drain_inst.ins.sync_info
        waits = list(si.on_wait) if si and si.on_wait else []
        if len(waits) > DRAIN_WAITS:
            si.on_wait = waits[:DRAIN_WAITS]
            rest = waits[DRAIN_WAITS:]
            for i in range(0, len(rest), DRAIN_WAITS):
                extra = nc.sync.drain()
                esi = extra.ins.sync_info
                chunk = rest[i : i + DRAIN_WAITS]
                if esi is None:
                    extra.ins.sync_info = type(si)(on_wait=chunk, on_update=[])
                else:
                    esi.on_wait = chunk
        nc.all_engine_barrier()
        popped = nc._tile_sem_poison_stack.pop()
        assert popped is self._sem_poison
        nc.clear_and_free_semaphores(list(self.sems.allocated().values()))
        nc.all_engine_barrier()

    tile.TileContext._drain_and_barrier = _drain_and_barrier
    tile.TileContext._acnn_drain_patched = True


def _split_multi_waits(nc, max_waits=_MAX_CTRL_WAITS):
    """This walrus build rejects >1 sem wait per compute instruction; hoist
    overflow waits onto same-engine NoOps placed just before (engine queues
    are strict FIFO, so the gating is equivalent)."""
    n_split = 0
    for f in nc.m.functions:
        for bb in f.blocks:
            new = []
            for inst in bb.instructions:
                si = inst.sync_info
                waits = list(si.on_wait) if si is not None and si.on_wait else []
                if len(waits) > max_waits:
                    extra = waits[: len(waits) - max_waits]
                    si.on_wait = waits[len(waits) - max_waits:]
                    for i in range(0, len(extra), max_waits):
                        nop = mybir.InstNoOp(
                            name=f"{inst.name}-xw{i}",
                            engine=inst.engine,
                            sync_info=mybir.SyncInfo(
                                on_wait=extra[i : i + max_waits], on_update=[]
                            ),
                            bass_nofuse=True,
                            ins=[],
                            outs=[],
                        )
                        new.append(nop)
                    n_split += 1
                new.append(inst)
            bb.instructions[:] = new
    return n_split


def build():
    _patch_tile_drain()
    nc = bass.Bass(trn_type="TRN2")

    AXW = 516  # per-f-chunk region width in the ax tiles (512 + shift fringe)
    XQW = T    # per-f-chunk width of the fp8 x copy (no halo; qkp reads
               # full token columns only). 4096 B stride: 16B-aligned for
               # the DoubleRow moving AP.

    xT = nc.dram_tensor("xT", [F, T], BF16, kind="ExternalInput")
    xQ = nc.dram_tensor("xQ", [128, NB * FC * TB], F8, kind="ExternalInput")
    wq = nc.dram_tensor("wq", [128, FC * H], F8, kind="ExternalInput")
    wk = nc.dram_tensor("wk", [128, FC * H], F8, kind="ExternalInput")
    cw = nc.dram_tensor("cw", [128, W * FC * H], BF16, kind="ExternalInput")
    vT = nc.dram_tensor("vT", [128, HC], BF16, kind="ExternalInput")
    ident = nc.dram_tensor("ident", [128, 128], BF16, kind="ExternalInput")
    # [NB*128, 4*H] bf16; token (blk*512 + g*128 + p) lives at row
    # (blk*128 + p), cols [g*512, (g+1)*512). Host de-interleaves + adds b.
    out = nc.dram_tensor("out", [NB * 128, 4 * H], BF16, kind="ExternalOutput")

    with ExitStack() as ctx:
        tc = ctx.enter_context(tile.TileContext(nc))

        const = ctx.enter_context(tc.tile_pool(name="const", bufs=1))
        wq_sb = const.tile([128, FC * H], F8, tag="wq")
        wk_sb = const.tile([128, FC * H], F8, tag="wk")
        cw_sb = const.tile([128, W * FC * H], BF16, tag="cw")
        vT_sb = const.tile([128, HC], BF16, tag="vT")
        id_sb = const.tile([128, 128], BF16, tag="ident")
        xt_sb = const.tile([128, FC * XW], BF16, tag="xt")
        xq_sb = const.tile([128, NB * FC * TB], F8, tag="xq")
        kp_sb = [const.tile([128, XW], BF16, name=f"kp{hc}", tag=f"kp{hc}") for hc in range(HC)]

        def dma_xq(b0, b1):
            # fp8 x for the q/k projections, block-major layout so each
            # partition's slice is (b1-b0)*2KB contiguous
            nc.sync.dma_start(
                xq_sb[:, b0 * FC * TB: b1 * FC * TB],
                xQ[:, b0 * FC * TB: b1 * FC * TB],
            )

        def dma_xt(fc, seg, half=None):
            o0, o1 = (0, S) if half is None else (half * (S // 2), (half + 1) * (S // 2))
            nc.sync.dma_start(
                xt_sb[:, fc * XW + seg * SEGW + PAD + o0: fc * XW + seg * SEGW + PAD + o1],
                xT[fc * 128:(fc + 1) * 128, seg * S + o0: seg * S + o1],
            )

        # issue order matters: the first qkp matmuls need xq(b0..b3) + wq.
        dma_xq(0, 1)
        nc.scalar.dma_start(wq_sb[:], wq[:])
        dma_xq(1, 4)
        nc.scalar.dma_start(wk_sb[:], wk[:])
        dma_xq(4, NB)
        nc.sync.dma_start(vT_sb[:], vT[:])
        nc.sync.dma_start(id_sb[:], ident[:])
        nc.sync.dma_start(cw_sb[:], cw[:])
        for fc in range(FC):
            dma_xt(fc, 0, 0)
            dma_xt(fc, 0, 1)
        for fc in range(FC):
            dma_xt(fc, 1)
        # zero the halo columns of x^T and kp^T
        for seg in range(BPC):
            for fc in range(FC):
                b0 = fc * XW + seg * SEGW
                nc.vector.memset(xt_sb[:, b0:b0 + PAD], 0.0)
                nc.vector.memset(xt_sb[:, b0 + PAD + S:b0 + SEGW], 0.0)
            for hc in range(HC):
                nc.vector.memset(kp_sb[hc][:, seg * SEGW: seg * SEGW + PAD], 0.0)
                nc.vector.memset(kp_sb[hc][:, seg * SEGW + PAD + S: (seg + 1) * SEGW], 0.0)

        # sc and tp share one pool/bank: scores -> exp -> transpose are strictly
        # sequential, freeing a bank so qkp gets 3
        qkp_ps = ctx.enter_context(tc.tile_pool(name="qkp_ps", bufs=4, space="PSUM"))
        sc_ps_pool = ctx.enter_context(tc.tile_pool(name="sc_ps", bufs=1, space="PSUM"))
        out_ps_pool = ctx.enter_context(tc.tile_pool(name="out_ps", bufs=3, space="PSUM"))

        qp = ctx.enter_context(tc.tile_pool(name="qp", bufs=2))
        argp = ctx.enter_context(tc.tile_pool(name="argp", bufs=2))
        thp = ctx.enter_context(tc.tile_pool(name="thp", bufs=5))
        smp = ctx.enter_context(tc.tile_pool(name="smp", bufs=4))
        attp = ctx.enter_context(tc.tile_pool(name="attp", bufs=2))
        flp = ctx.enter_context(tc.tile_pool(name="flp", bufs=2))
        bcp = ctx.enter_context(tc.tile_pool(name="bcp", bufs=2))
        axp = ctx.enter_context(tc.tile_pool(name="axp", bufs=7))
        outp = ctx.enter_context(tc.tile_pool(name="outp", bufs=2))

        def xcol0(b):
            return (b // NBSEG) * SEGW + PAD + (b % NBSEG) * TB

        q_tiles = {}
        th_tiles = {}
        attT_tiles = {}
        ab_tiles = {}
        ax_tiles = {}
        rc_tiles = {}
        ex_tiles = {}

        xq4 = xq_sb[:].rearrange("p (b f t) -> p b f t", b=NB, f=FC)
        wq3 = wq_sb[:].rearrange("p (f h) -> p f h", f=FC)
        wk3 = wk_sb[:].rearrange("p (f h) -> p f h", f=FC)

        def emit_qkp(b):
            qt = qp.tile([128, HC * TB], BF16, name="qt", tag="qt")
            q_tiles[b] = qt
            for hc in range(HC):
                for which, w3 in ((0, wq3), (1, wk3)):
                    ps = qkp_ps.tile([128, TB], F32, name="ps", tag="qkp")
                    for pi in range(2):
                        # fp8 DoubleRow: contraction 256 per matmul
                        nc.tensor.matmul(
                            ps[:],
                            w3[:, 2 * pi:2 * pi + 2, hc * 128:(hc + 1) * 128],
                            xq4[:, b, 2 * pi:2 * pi + 2, :],
                            start=(pi == 0),
                            stop=(pi == 1),
                            perf_mode=DR,
                        )
                    if which == 0:
                        nc.scalar.activation(qt[:, hc * TB:(hc + 1) * TB], ps[:], AF.Copy)
                    else:
                        nc.vector.tensor_copy(kp_sb[hc][:, xcol0(b):xcol0(b) + TB], ps[:])

        def emit_addstanh(b):
            xc = xcol0(b)
            qt = q_tiles.pop(b)
            ths = []
            for hc in range(HC):
                arg = argp.tile([128, W * TB], BF16, name="arg", tag="arg")
                for w in range(W):
                    nc.vector.tensor_add(
                        arg[:, w * TB:(w + 1) * TB],
                        kp_sb[hc][:, xc - PAD + w: xc - PAD + w + TB],
                        qt[:, hc * TB:(hc + 1) * TB],
                    )
                th = thp.tile([128, W * TB], BF16, name="th", tag="th")
                # q/kp carry a 64x fp8 weight pre-scale; descale inside tanh
                nc.scalar.activation(th[:], arg[:], AF.Tanh, scale=1.0 / WSCALE)
                ths.append(th)
            th_tiles[b] = ths

        def emit_scores(b):
            sc_ps = sc_ps_pool.tile([128, 32], F32, name="sc", tag="sc")
            ths = th_tiles.pop(b)
            for hc in range(HC):
                th = ths[hc]
                for w in range(W):
                    for g in range(4):
                        col = g * 8 + w
                        nc.tensor.matmul(
                            sc_ps[:, col:col + 1],
                            th[:, w * TB + g * 128: w * TB + (g + 1) * 128],
                            vT_sb[:, hc:hc + 1],
                            start=(hc == 0 and w == 0 and g == 0),
                            stop=(hc == HC - 1 and w == W - 1 and g == 3),
                            skip_group_check=True,
                        )
            # softmax over W=5 (scores bounded by |v|_1 ~ 8, no max-sub
            # needed). The broadcastable row tensor carries UNNORMALIZED
            # exp(scores); 1/sum folds into the cnn PSUM drain as a
            # per-partition ACT scale two iterations later, so the PE
            # transposes depend on nothing but the exp.
            attT = attp.tile([W, TB], BF16, name="attT", tag="attT")
            ex = smp.tile([128, 4 * W], BF16, name="ex", tag="ex")
            nc.scalar.activation(
                ex[:].rearrange("p (g w) -> p g w", g=4),
                sc_ps[:].rearrange("p (g w) -> p g w", g=4)[:, :, 0:W],
                AF.Exp,
            )
            tp = sc_ps_pool.tile([128, TB], BF16, name="tp", tag="sc")
            for g in range(4):
                nc.tensor.transpose(
                    tp[0:W, g * 128:(g + 1) * 128], ex[:, g * W:(g + 1) * W], id_sb[:]
                )
            nc.scalar.activation(attT[:], tp[0:W, :], AF.Copy)
            ex_tiles[b] = ex
            attT_tiles[b] = attT

        def emit_sumrecip(b):
            # deferred to the head of the following iteration: keeps DVE's
            # FIFO from idling at the iteration boundary (rc is only consumed
            # by the cnn drain two iterations later)
            ex = ex_tiles.pop(b)
            sm = smp.tile([128, 4], F32, name="sm", tag="sm")
            nc.vector.reduce_sum(sm[:], ex[:].rearrange("p (g w) -> p g w", g=4), AX.X)
            rc = smp.tile([128, 4], F32, name="rc", tag="rc")
            nc.vector.reciprocal(rc[:], sm[:])
            rc_tiles[b] = rc

        def emit_bcast(b):
            # two-hop att broadcast with fanned-out source reads:
            #   hop 1: attT row w -> cols [ABST*w, ABST*w+TB) of FOUR
            #          replica partitions (5 DMAs x 4 descriptors)
            #   hop 2: replica k -> dst partition quadrant k (4 DMAs x 32
            #          descriptors of 5.2KB, each quadrant reading its own
            #          source partition's port)
            # 148 descriptors/block instead of 640, without the
            # single-partition source port bottleneck of a 1->128 fan-out.
            attT = attT_tiles.pop(b)
            eng = nc.sync
            fl = flp.tile([64, FLW], BF16, name="fl", tag="fl")
            # hop 1a: flatten attT's 5 rows into replica row 0 (one dma,
            # 5 descriptors), then 1b: replicate row 0 to the other 7
            # port-spread rows. Two triggers instead of five.
            eng.dma_start(
                fl[0:1, :].rearrange("p (w j) -> p w j", w=W)[:, :, 0:TB],
                attT[0:W, :],
            )
            eng.dma_start(
                fl[8:64:8, 0:ABF],
                fl[0:1, 0:ABF].rearrange("p (r c) -> p r c", r=1)
                .broadcast_to((1, 7, ABF)),
            )
            # single broadcast whose 128 descriptors read from 8 source
            # partitions sitting on 8 different SBUF AXI ports (stride 8), so
            # the fan-out runs at ~8x the single-port read rate
            ab = bcp.tile([128, ABF], BF16, name="ab", tag="ab")
            eng.dma_start(
                ab[:],
                fl[0:64:8, 0:ABF].rearrange("p (r c) -> p r c", r=1)
                .broadcast_to((8, 16, ABF)),
            )
            ab_tiles[b] = ab

        def emit_ax(b):
            xc = xcol0(b)
            ab = ab_tiles.pop(b)
            axs = []
            for w in range(W):
                ax = axp.tile([128, FC * AXW], BF16, name="ax", tag="ax")
                # ax[:, fc, j] = xt[:, fc, xc-2+j] * att_w[j-w]   (j in [0,516))
                nc.vector.tensor_tensor(
                    ax.rearrange("p (f c) -> p f c", f=FC)[:, :, 0:516],
                    xt_sb.rearrange("p (f c) -> p f c", f=FC)[:, :, xc - PAD: xc - PAD + 516],
                    ab[:, ABW * w: ABW * w + 516].rearrange("p (r c) -> p r c", r=1)
                    .broadcast_to((128, FC, 516)),
                    mybir.AluOpType.mult,
                )
                axs.append(ax)
            ax_tiles[b] = axs

        def emit_cnnmm(b):
            axs = ax_tiles.pop(b)
            rc = rc_tiles.pop(b)
            ot = outp.tile([128, 4 * H], BF16, name="ot", tag="ot")
            # psum [token-group, h]; g-outer so each group's psum closes early
            # and the ACT drain (scaled by 1/softmax-sum per token partition)
            # spreads across the iteration instead of bunching at its end
            for g in range(4):
                op = out_ps_pool.tile([128, H], F32, name=f"op{g}", tag="op")
                for w in range(W):
                    ax = axs[w]
                    for fc in range(FC):
                        nc.tensor.matmul(
                            op[:],
                            ax[:, fc * AXW + w + g * 128: fc * AXW + w + (g + 1) * 128],
                            cw_sb[:, (w * FC + fc) * H:(w * FC + fc + 1) * H],
                            start=(w == 0 and fc == 0),
                            stop=(w == W - 1 and fc == FC - 1),
                        )
                nc.scalar.activation(ot[:, g * H:(g + 1) * H], op[:], AF.Copy, scale=rc[:, g:g + 1])
            # one 4KB-per-partition store per block; tail-store variants
            # (per-group early stores, SWDGE final half) all measure within
            # noise -- the final store's latency hides under the exit drain
            nc.sync.dma_start(out[b * 128:(b + 1) * 128, :], ot[:])

        # Software-pipelined emission. Per-engine FIFO program order within a
        # steady-state iteration:
        #   PE : qkp(b+4) | cnn-matmuls(b) | scores+transposes(b+2)
        #   DVE: args(b+2) | ax-mults(b+1) | softmax bits(b+2) | out-add(b)
        #   ACT: tanh(b+2) | q-copy(b+4) | exp/attT(b+2)
        # so by the time PE reaches scores(b+2) the tanh chain has had a full
        # cnn's worth of slack, and cnn(b+1)'s ax tiles were multiplied one
        # iteration ahead (broadcast DMA latency hidden behind cnn(b)).
        emit_qkp(0)
        emit_qkp(1)
        emit_addstanh(0)
        emit_qkp(2)
        emit_qkp(3)
        emit_scores(0)
        emit_bcast(0)
        emit_sumrecip(0)
        emit_addstanh(1)
        emit_scores(1)
        emit_bcast(1)
        emit_sumrecip(1)
        emit_addstanh(2)
        # qkp(4) pulled into the preamble: the PE otherwise idles here
        # waiting on the attention chain, and qt(0..2) are already
        # consumed so the 2-buffer qt pool has room
        emit_qkp(4)
        emit_ax(0)
        # steady state: scores(b+2) run at the START of iter b (their tanh ran
        # last iter), so attT/bcast fire ~20us before ax(b+2) needs them and
        # the attention chain never gates the PE
        for b in range(NB):
            if b + 4 < NB and b > 0:
                emit_qkp(b + 4)
            if b + 2 < NB:
                emit_scores(b + 2)
                emit_bcast(b + 2)
                emit_sumrecip(b + 2)
            if b + 3 < NB:
                emit_addstanh(b + 3)
            if b + 1 < NB:
                emit_ax(b + 1)
            emit_cnnmm(b)

    _split_multi_waits(nc)
    return nc


_NC_CACHE = None


def _get_nc():
    global _NC_CACHE
    if _NC_CACHE is None:
        _NC_CACHE = build()
    return _NC_CACHE


def _prep_in_maps(embeds_output, Wq, Wk, v_att, cnn_W):
    bf = ml_dtypes.bfloat16
    f8 = ml_dtypes.float8_e4m3
    # wq/wk: fp8, pre-scaled by WSCALE, packed [128, FC*H]
    # (partition p holds concat over fc of W[fc*128+p, :])
    wqs = np.asarray(Wq, dtype=np.float32) * WSCALE
    wks = np.asarray(Wk, dtype=np.float32) * WSCALE
    wq8 = np.ascontiguousarray(
        wqs.reshape(FC, 128, H).transpose(1, 0, 2).reshape(128, FC * H)
    ).astype(f8)
    wk8 = np.ascontiguousarray(
        wks.reshape(FC, 128, H).transpose(1, 0, 2).reshape(128, FC * H)
    ).astype(f8)
    # cnn_W: bf16, packed [128, W*FC*H]
    cwf = np.asarray(cnn_W, dtype=np.float32)
    cwp = np.ascontiguousarray(
        cwf.reshape(W * FC, 128, H).transpose(1, 0, 2).reshape(128, W * FC * H)
    ).astype(bf)
    vTp = np.ascontiguousarray(
        np.asarray(v_att, dtype=np.float32).reshape(HC, 128).T
    ).astype(bf)
    identp = np.eye(128, dtype=np.float32).astype(bf)

    x = np.asarray(embeds_output, dtype=np.float32)
    in_maps = []
    for c in range(NCORES):
        shard = x[c * BPC:(c + 1) * BPC]                  # [BPC, S, F]
        xTs = shard.transpose(2, 0, 1).reshape(F, T)      # [F, BPC*S]
        xTb = np.ascontiguousarray(xTs).astype(bf)
        # fp8 copy in [128, FC*T] layout, quantized from the bf16 values
        # the device sees (matches the accuracy sim)
        xq8 = np.ascontiguousarray(
            xTb.astype(np.float32).reshape(FC, 128, NB, TB)
            .transpose(1, 2, 0, 3).reshape(128, NB * FC * TB)
        ).astype(f8)
        in_maps.append(
            {
                "xT": xTb,
                "xQ": xq8,
                "wq": wq8,
                "wk": wk8,
                "cw": cwp,
                "vT": vTp,
                "ident": identp,
            }
        )
    return in_maps


def kernel(embeds_output, Wq, Wk, v_att, cnn_W, cnn_b, **run_kwargs):
    nc = _get_nc()
    in_maps = _prep_in_maps(embeds_output, Wq, Wk, v_att, cnn_W)
    res = run_bass_kernel_spmd(nc, in_maps, core_ids=list(range(NCORES)), **run_kwargs)
    # device output is [NB*128, 4*H] bf16 per core: token blk*512+g*128+p at
    # (blk*128+p, g*512:+512). De-interleave + upcast + add bias here.
    shards = []
    for c in range(NCORES):
        o = np.asarray(res.results[c]["out"], dtype=np.float32)
        o = o.reshape(NB, 128, 4, H).transpose(0, 2, 1, 3).reshape(BPC, S, H)
        shards.append(o)
    full = np.concatenate(shards, axis=0) + np.asarray(cnn_b, dtype=np.float32)
    kernel.last_results = res
    return full
